# revision 25
# baseline (speedup 1.0000x reference)
"""MiniRocketFeatures Trainium2 Bass kernel.

Strategy (data-parallel over batch, 8 batches per core x 8 cores):

The MiniRocket per-dilation pipeline is
    C[k,t] = sum_{c,j} (W[k,j] * comb[c,k]) * x_pad[c, t + j*d]     (conv+channel-combine)
    PPV[k,f] = mean_t (C[k,t] > bias[k,f])                          (two parity halves,
                                                                     odd half center-cropped)
We fold conv + channel-combine into ONE 90-row matmul per output tile:
  - row j*10+c (c<9) of the moving operand is tap j of channel c: a shifted
    window of a zero-padded x copy.  All 90 rows come from a single strided DMA
    (DRAM-side access pattern [j: step d][c: step 4088][t: step 1] - the tap
    shift is linear in j, so no on-chip data shuffling is needed).
  - rows j*10+9 read a 0/1 indicator channel z (1 in the padded margins).  Taps
    j=0 / j=8 of z are exactly the left/right crop margins for ANY dilation;
    their lhsT coefficients are -1e30 for the center-cropped parity half (0 for
    the other taps), which forces C = -1e30 outside the crop window.  Counts
    for the cropped half can then be taken over the full 2048 samples: is_gt
    contributes 0, Sign contributes -1 (absorbed by a host-side affine).
PPV counting runs as fused compare+reduce single-pass ops:
  - VectorE: tensor_scalar(op0=is_gt, op1=add, accum_out)  (2 elem/cycle fp32 SBUF)
  - ScalarE: activation(Sign, bias=-b, accum_out), count = (acc + 2048) / 2
ScalarE also evacuates PSUM->SBUF (it is closer to PSUM; DMA cannot touch PSUM).
The kernel returns raw per-(k, b, col) accumulators; the host normalizes by the
(parity-dependent) window length and scatters into the reference feature order.
"""

import ml_dtypes
import numpy as np

import concourse.bass as bass
import concourse.bacc as bacc
import concourse.tile as tile
from concourse import mybir
from concourse import bass_utils

# ---------------------------------------------------------------- schedule ----
KS = 9
NK = 84
CIN = 9
L = 2048
B = 64
NCORES = 8
BPC = B // NCORES  # batches per core

DILATIONS = [1, 2, 3, 4, 5, 7, 8, 10, 12, 14, 17, 20, 25, 29, 35, 42, 51, 61,
             73, 87, 104, 125, 149, 178, 213, 255]
NFPD = [15, 12, 4, 4, 8, 4, 4, 4, 4, 4, 4, 4, 4, 4, 4, 4, 4, 4, 3, 3, 3, 3, 3,
        3, 3, 3]
ND = len(DILATIONS)
NCOL = sum(NFPD)            # 119 (i, f) compare columns
XPAD = 1020                 # max padding = 4*255
XLEN = XPAD + L + XPAD      # 4088
NCH = CIN + 1               # 9 x-channels + 1 margin-indicator channel
MARGIN_VAL = -1.0e30
# Dilations with short crop windows run in fp32: a single bf16-induced compare
# flip there costs 1/L_crop (= 0.125 at d=255) in the output.
F32_DILS = {24, 25}

# column -> dilation index
COL_I = np.repeat(np.arange(ND), NFPD)            # [119]
COL_F = np.concatenate([np.arange(f) for f in NFPD])

# Engine assignment per column: ~9% of compares on ScalarE (it also does all
# PSUM evacuations), rest on VectorE.  Bresenham spread in execution order.
ACT_FRAC = 0.09
ASSIGN_ACT = np.zeros(NCOL, dtype=bool)

# Execution order: interleave compare-heavy (F=15/12/8) and evac-heavy (F=3)
# dilations so VectorE and ScalarE loads stay balanced through the pipeline.
EXEC_ORDER = [0, 18, 19, 1, 20, 21, 4, 22, 23, 2, 24, 3, 25, 5, 6, 7, 8, 9,
              10, 11, 12, 13, 14, 15, 16, 17]
COL_BASE = np.concatenate([[0], np.cumsum(NFPD)[:-1]])

_err = 0.0
for _i in EXEC_ORDER:
    for _f in range(NFPD[_i]):
        _err += ACT_FRAC
        if _err >= 1.0:
            ASSIGN_ACT[COL_BASE[_i] + _f] = True
            _err -= 1.0

_CACHE = {}


def _build_module():
    nc = bacc.Bacc("TRN2", target_bir_lowering=False, debug=False,
                   num_devices=NCORES)
    xz_d = nc.dram_tensor("xz", [BPC, NCH, XLEN], mybir.dt.bfloat16,
                          kind="ExternalInput").ap()
    xz32_d = nc.dram_tensor("xz32", [BPC, NCH, XLEN], mybir.dt.float32,
                            kind="ExternalInput").ap()
    wts_d = nc.dram_tensor("wts", [90, ND, NK], mybir.dt.bfloat16,
                           kind="ExternalInput").ap()
    wts32_d = nc.dram_tensor("wts32", [90, ND, NK], mybir.dt.float32,
                             kind="ExternalInput").ap()
    bias_d = nc.dram_tensor("bias_eff", [NK, NCOL], mybir.dt.float32,
                            kind="ExternalInput").ap()
    accD_d = nc.dram_tensor("accD", [NK, BPC * NCOL], mybir.dt.float32,
                            kind="ExternalOutput").ap()
    accA_d = nc.dram_tensor("accA", [NK, BPC * NCOL], mybir.dt.float32,
                            kind="ExternalOutput").ap()

    with tile.TileContext(nc) as tc:
        with tc.tile_pool(name="res", bufs=1) as res, \
             tc.tile_pool(name="gxp", bufs=6) as gxp, \
             tc.tile_pool(name="cp", bufs=12) as cp, \
             tc.tile_pool(name="psp", bufs=2, space="PSUM") as psp:
            lhsT = res.tile([90, ND, NK], mybir.dt.bfloat16)
            lhsT32 = res.tile([90, ND, NK], mybir.dt.float32)
            bias_sb = res.tile([NK, NCOL], mybir.dt.float32)
            accD_sb = res.tile([NK, BPC * NCOL], mybir.dt.float32)
            accA_sb = res.tile([NK, BPC * NCOL], mybir.dt.float32)
            junkD = res.tile([NK, L], mybir.dt.bfloat16)
            junkA = res.tile([NK, L], mybir.dt.bfloat16)

            nc.sync.dma_start(out=lhsT, in_=wts_d)
            nc.sync.dma_start(out=lhsT32, in_=wts32_d)
            nc.sync.dma_start(out=bias_sb, in_=bias_d)
            nc.vector.memset(accD_sb, 0.0)
            nc.scalar.memzero(accA_sb)

            for i in EXEC_ORDER:
                d = DILATIONS[i]
                col_base = int(COL_BASE[i])
                F = NFPD[i]
                f32 = i in F32_DILS
                src_t = xz32_d if f32 else xz_d
                gx_dt = mybir.dt.float32 if f32 else mybir.dt.bfloat16
                for b in range(BPC):
                    if f32:
                        gx = gxp.tile([90, L], gx_dt, tag="gx32", bufs=3)
                    else:
                        gx = gxp.tile([90, L], gx_dt, tag="gx")
                    # rows (j*10+c): x_pad[b, c, (1020 - 4d + j*d) + t];
                    # channel 9 is the margin indicator z (only taps 0/8 used)
                    src_main = bass.AP(
                        tensor=src_t.tensor,
                        offset=b * NCH * XLEN + XPAD - 4 * d,
                        ap=[[d, KS], [XLEN, NCH], [1, L]])
                    nc.sync.dma_start(out=gx, in_=src_main)
                    ps = psp.tile([NK, L], mybir.dt.float32)
                    for n in range(4):
                        nc.tensor.matmul(
                            ps[:, n * 512:(n + 1) * 512],
                            (lhsT32 if f32 else lhsT)[:, i, :],
                            gx[:, n * 512:(n + 1) * 512],
                            start=True, stop=True)
                    if f32:
                        cbt = cp.tile([NK, L], mybir.dt.float32, tag="c32",
                                      bufs=4)
                    else:
                        cbt = cp.tile([NK, L], mybir.dt.bfloat16, tag="c",
                                      bufs=14)
                    nc.scalar.activation(
                        out=cbt, in_=ps,
                        func=mybir.ActivationFunctionType.Copy)
                    for f in range(F):
                        col = col_base + f
                        a_slot = b * NCOL + col
                        cslice = cbt
                        if ASSIGN_ACT[col]:
                            # defer sign ops behind evacuations on ScalarE; the
                            # c-tile pool provides backpressure
                            with tc.high_priority(offset=-100000):
                                nc.scalar.activation(
                                    out=junkA, in_=cslice,
                                    func=mybir.ActivationFunctionType.Sign,
                                    bias=bias_sb[:, col:col + 1], scale=1.0,
                                    accum_out=accA_sb[:, a_slot:a_slot + 1])
                        else:
                            nc.vector.tensor_scalar(
                                out=junkD, in0=cslice,
                                scalar1=bias_sb[:, col:col + 1], scalar2=None,
                                op0=mybir.AluOpType.is_gt,
                                op1=mybir.AluOpType.add,
                                accum_out=accD_sb[:, a_slot:a_slot + 1])

            nc.sync.dma_start(out=accD_d, in_=accD_sb)
            nc.sync.dma_start(out=accA_d, in_=accA_sb)

    nc.compile()
    return nc


def _host_tables(kernels, channel_combinations, biases):
    """lhsT weights [83, ND, 84], effective bias [84, NCOL] (negated for ACT cols)."""
    kern = np.asarray(kernels, dtype=np.float32).reshape(CIN, NK, KS)  # [c,k,j]
    comb = np.asarray(channel_combinations, dtype=np.float32)[..., 0]  # [i,c,k]
    bias = np.asarray(biases, dtype=np.float32)                        # [i,k,maxF]

    wts = np.zeros((90, ND, NK), dtype=np.float32)
    k_par = (np.arange(NK) % 2)
    rows = np.arange(90)
    xrows = rows[rows % 10 != 9]                            # rows carrying x
    for i in range(ND):
        # row j*10+c, col k : kern[c,k,j] * comb[i,c,k]; row j*10+9 reads z
        m = kern.transpose(2, 0, 1) * comb[i][None, :, :]   # [j, c, k]
        wts[xrows, i, :] = m.reshape(81, NK)
        cropped = (k_par != (i % 2))                        # cropped-parity kernels
        wts[0 * 10 + 9, i, cropped] = MARGIN_VAL            # z tap j=0: left margin
        wts[8 * 10 + 9, i, cropped] = MARGIN_VAL            # z tap j=8: right margin
    wts_bf = wts.astype(ml_dtypes.bfloat16)

    bias_eff = np.empty((NK, NCOL), dtype=np.float32)
    for col in range(NCOL):
        v = bias[COL_I[col], :, COL_F[col]]
        bias_eff[:, col] = -v if ASSIGN_ACT[col] else v
    return wts_bf, wts, bias_eff


def _out_index_and_len():
    """OUTIDX [84, NCOL] position in the 9996-wide output; LHALF [84, NCOL]."""
    outidx = np.zeros((NK, NCOL), dtype=np.int64)
    lhalf = np.zeros((NK, NCOL), dtype=np.float32)
    base = 0
    col = 0
    ks = np.arange(NK)
    for i, d in enumerate(DILATIONS):
        F = NFPD[i]
        p1 = i % 2
        full = (ks % 2 == p1)
        lcrop = L - 8 * d
        for f in range(F):
            outidx[full, col] = base + (ks[full] // 2) * F + f
            outidx[~full, col] = base + 42 * F + (ks[~full] // 2) * F + f
            lhalf[full, col] = L
            lhalf[~full, col] = lcrop
            col += 1
        base += 2 * 42 * F
    return outidx, lhalf


def kernel(x, kernels, channel_combinations, biases):
    x = np.asarray(x, dtype=np.float32)
    wts_bf, wts32, bias_eff = _host_tables(kernels, channel_combinations, biases)

    # padded x + margin-indicator channel z
    xz32 = np.zeros((B, NCH, XLEN), dtype=np.float32)
    xz32[:, 0:CIN, XPAD:XPAD + L] = x
    xz32[:, CIN, :XPAD] = 1.0
    xz32[:, CIN, XPAD + L:] = 1.0
    xz = xz32.astype(ml_dtypes.bfloat16)

    if "nc" not in _CACHE:
        _CACHE["nc"] = _build_module()
    nc = _CACHE["nc"]

    in_maps = []
    for c in range(NCORES):
        in_maps.append({
            "xz": np.ascontiguousarray(xz[c * BPC:(c + 1) * BPC]),
            "xz32": np.ascontiguousarray(xz32[c * BPC:(c + 1) * BPC]),
            "wts": wts_bf,
            "wts32": wts32,
            "bias_eff": bias_eff,
        })
    res = bass_utils.run_bass_kernel_spmd(nc, in_maps,
                                          core_ids=list(range(NCORES)))

    outidx, lhalf = _out_index_and_len()
    assign = ASSIGN_ACT[None, None, :]                     # [1,1,NCOL]
    out = np.empty((B, 84 * NCOL), dtype=np.float32)
    for c in range(NCORES):
        accD = res.results[c]["accD"].reshape(NK, BPC, NCOL)
        accA = res.results[c]["accA"].reshape(NK, BPC, NCOL)
        counts = np.where(assign, (accA + float(L)) * 0.5, accD)
        ppv = counts / lhalf[:, None, :]                   # [84, BPC, NCOL]
        for b in range(BPC):
            out[c * BPC + b, outidx.ravel()] = ppv[:, b, :].ravel()
    return out
